# revision 18
# baseline (speedup 1.0000x reference)
"""MCorr1d Trainium2 kernel (8 NeuronCores).

Problem (hardcoded from spec):
  in_    [1024, 64, 512]  fp32   (X, N, C_in)
  weight [16, 512, 512]   fp32   (KW, C_in, C_out)
  bias   [512]            fp32
  out    [64, 64, 512]    fp32   (Y, N, C_out)

  out[y, n, o] = bias[o] + sum_{w=0}^{15} sum_c in_[(y+1)*(w+1)-1, n, c] * weight[w, c, o]

Sharding: NSH-way data-parallel over N x CSH-way tensor-parallel over
C_out (NSH*CSH = 8). Default 4x2: core (nq, oq) computes rows
r = y*16 + n_local (1024 rows) for couts oq*256..oq*256+255.

Why 4x2 and not 8x1: the PE does NOT overlap the 128-cycle stationary
(ldweights) load with the moving pass, so per-core cycles =
moving_total + 128*n_loads. With 1024 rows per core, each stationary
W chunk [128k, 128o] serves TWO consecutive 512-row matmuls (row
halves, one PSUM bank each); stripping the redundant second Ldweights
(legalized matmuls carry "ldweights": false and reuse the loaded
array) halves the load count: 16w*4k*2o*(2*512 + 128) cycles = 147k
vs 8x1's 164k.

W is the stationary operand; A streams 512-row halves; out accumulates
in [128o, 512r] PSUM tiles (OC*RH = 4 banks x ps bufs=2 = all 8).
Bias is added by the Activation engine during the PSUM->SBUF drain
(per-partition bias AP), not by PE matmuls.

Host packs A/W p-major ([KW, 128, KC*fd]) so each DMA partition line is
one contiguous read. A tiles stream on the SP HWDGE queue, W tiles on
the Activation HWDGE queue.

Precision modes:
  bf16x3 : hi/lo bf16 split, 3 matmuls (hi@hi + hi@lo + lo@hi), rel err ~5e-6
  bf16   : plain bf16 (half DMA bytes, 1 cycle/row), rel err ~2e-3
"""

import contextlib
import json

import numpy as np

X_LEN, N_BATCH, C_IN = 1024, 64, 512
KW, C_OUT = 16, 512
Y_OUT = 64
N_CORES = 8
KC = C_IN // 128  # 4 k-chunks

MODE = "bf16"
NSH, CSH = 4, 2          # batch x cout sharding, NSH*CSH == 8
AB_BUFS = 6              # taps in flight per A/W tile ring
PS_BUFS = 2              # PSUM bank sets (2 = overlap drain with next iter)
W_QUEUE = "scalar"       # HWDGE queue for W tiles: "scalar" (Act) or "sync" (SP)
STRIP_LDW = True         # drop redundant Ldweights for shared stationaries
MERGE_LDW = False       # fold kept Ldweights into self-loading Matmults
FP8_TAPS = 2             # taps 0..n-1 run fp8e4 DoubleRow (2x PE rate incl.
                         # halved Ldweights); err ~ sqrt(n/16)*3.8e-2 + 2.4e-3
SA, SW = (32.0, 16384.0) if FP8_TAPS else (1.0, 1.0)  # e4m3 scale-up; both
                         # powers of 2 so bf16 taps scale exactly; the drain
                         # descales via the Activation `scale` operand

N_PER = N_BATCH // NSH   # batch rows per core
ROWS = Y_OUT * N_PER     # output rows per core
COUTS = C_OUT // CSH     # output channels per core
OC = COUTS // 128        # o-chunks per core
RH = ROWS // 512         # 512-row moving passes per stationary

_XS = np.array([[(y + 1) * (w + 1) - 1 for y in range(Y_OUT)] for w in range(KW)])


def _strip_redundant_ldweights(bir_json: bytes) -> bytes:
    """Remove back-to-back PE Ldweights with identical weight APs.

    Legalization emits one Ldweights per Matmult even when consecutive
    matmuls share the stationary operand; the PE array retains its
    contents across matmuls ("ldweights": false), so the repeats only
    burn 128 PE cycles each. Keep any Ldweights that carries sync_info.
    """
    j = json.loads(bir_json)
    dropped = 0
    for fn in j["functions"]:
        for blk in fn["blocks"]:
            out = []
            last_ld = None
            for inst in blk["instructions"]:
                op = inst.get("opcode")
                if inst.get("engine") == "PE":
                    if op == "Ldweights":
                        key = json.dumps(inst["ins"], sort_keys=True)
                        sync = inst.get("sync_info") or {}
                        clean = not sync.get("on_wait") and not sync.get("on_update")
                        if key == last_ld and clean:
                            dropped += 1
                            continue
                        last_ld = key
                    elif op == "Matmult":
                        pass  # array contents persist across matmuls
                    else:
                        last_ld = None
                out.append(inst)
            blk["instructions"] = out
    return json.dumps(j).encode()


def _merge_ldweights(bir_json: bytes) -> bytes:
    """Fold each PE Ldweights into the immediately following Matmult on the
    same weights AP, making it self-loading ("ldweights": true) and merging
    any semaphore waits. Halves the PE instruction count per stationary."""
    j = json.loads(bir_json)
    for fn in j["functions"]:
        for blk in fn["blocks"]:
            insts = blk["instructions"]
            out = []
            i = 0
            while i < len(insts):
                inst = insts[i]
                if (inst.get("engine") == "PE"
                        and inst.get("opcode") == "Ldweights"
                        and i + 1 < len(insts)):
                    nxt = insts[i + 1]
                    if (nxt.get("engine") == "PE"
                            and nxt.get("opcode") == "Matmult"
                            and nxt.get("ldweights") is False
                            and json.dumps(nxt["ins"][-1], sort_keys=True)
                            == json.dumps(inst["ins"][0], sort_keys=True)):
                        ld_sync = inst.get("sync_info") or {}
                        mm_sync0 = nxt.get("sync_info") or {}
                        combined = (list(ld_sync.get("on_wait") or [])
                                    + list(mm_sync0.get("on_wait") or []))
                        # The TPB LW struct has limited wait slots; keep the
                        # separate Ldweights when merging would exceed one.
                        if not ld_sync.get("on_update") and len(combined) <= 1:
                            nxt = dict(nxt)
                            nxt["ldweights"] = True
                            if combined:
                                mm_sync = dict(mm_sync0)
                                mm_sync["on_wait"] = combined
                                mm_sync.setdefault("on_update", [])
                                nxt["sync_info"] = mm_sync
                            out.append(nxt)
                            i += 2
                            continue
                out.append(inst)
                i += 1
            blk["instructions"] = out
    return json.dumps(j).encode()


def _make_bacc():
    from concourse import bacc

    class _Bacc(bacc.Bacc):
        def to_json_bytes(self):
            data = super().to_json_bytes()
            if STRIP_LDW:
                data = _strip_redundant_ldweights(data)
            if MERGE_LDW:
                data = _merge_ldweights(data)
            return data

    return _Bacc("TRN2", target_bir_lowering=False, debug=False,
                 num_devices=N_CORES)


def _build_nc(mode):
    return _build_nc_reps(mode, 1)


def _build_nc_reps(mode, reps, loop_n=0):
    import concourse.mybir as mybir
    import concourse.tile as tile

    f32 = mybir.dt.float32
    if mode in ("bf16x3", "bf16"):
        mdt = mybir.dt.bfloat16
    else:
        raise ValueError(mode)

    nc = _make_bacc()

    # Per-core DRAM tensors (SPMD: same program, different data per core).
    ins = {}
    names = (("a_hi", "w_hi", "w_lo", "a_lo") if mode == "bf16x3"
             else ("a", "w"))
    fp8n = FP8_TAPS if mode == "bf16" else 0
    for nm in names:
        fd = ROWS if nm.startswith("a") else COUTS
        ins[nm] = nc.dram_tensor(nm, [KW - fp8n, 128, KC * fd], mdt,
                                 kind="ExternalInput").ap()
    if fp8n:
        f8 = mybir.dt.float8e4
        ins["a8"] = nc.dram_tensor("a8", [fp8n, 128, KC * ROWS], f8,
                                   kind="ExternalInput").ap()
        ins["w8"] = nc.dram_tensor("w8", [fp8n, 128, KC * COUTS], f8,
                                   kind="ExternalInput").ap()
    bias_t = nc.dram_tensor("bias", [128, OC], f32, kind="ExternalInput").ap()
    out_t = nc.dram_tensor("out", [COUTS, ROWS], f32, kind="ExternalOutput").ap()

    with tile.TileContext(nc) as tc:
        with tc.tile_pool(name="asb", bufs=AB_BUFS) as asb, \
             tc.tile_pool(name="csb", bufs=1) as csb, \
             tc.tile_pool(name="osb", bufs=2) as osb, \
             tc.tile_pool(name="ps", bufs=PS_BUFS, space="PSUM") as ps:

            bias_sb = csb.tile([128, OC], f32, tag="bias")
            nc.sync.dma_start(bias_sb[:], bias_t[:])

            loop_cm = (tc.For_i(0, loop_n, 1) if loop_n
                       else contextlib.nullcontext())
            with loop_cm:
                for _rep in range(reps):
                    _emit_body(nc, mode, mdt, f32, asb, osb, ps,
                               bias_sb, ins, out_t)

    nc.compile()
    return nc


def _emit_body(nc, mode, mdt, f32, asb, osb, ps, bias_sb, ins, out_t):
    import concourse.mybir as mybir

    # Accumulators: OC x RH PSUM banks of [128, 512]; bufs=2 on the pool
    # alternates bank sets across iterations so the drain of iteration i
    # overlaps the matmuls of iteration i+1.
    acc = [[ps.tile([128, 512], f32, name=f"acc{o}_{h}", tag=f"acc{o}_{h}")
            for h in range(RH)] for o in range(OC)]

    fp8n = FP8_TAPS if mode == "bf16" else 0
    f8 = mybir.dt.float8e4
    for w in range(KW):
        if w < fp8n:
            # fp8 DoubleRow tap: 2 k-chunks contract per matmul at 2 rows
            # per cycle; the stationary [128, 2, 128] load also halves.
            at = asb.tile([128, KC, ROWS], f8, name="a8_t", tag="a8")
            nc.sync.dma_start(at[:], ins["a8"][w])
            wt = asb.tile([128, KC, COUTS], f8, name="w8_t", tag="w8")
            eng = nc.sync if W_QUEUE == "sync" else nc.scalar
            eng.dma_start(wt[:], ins["w8"][w])
            for o in range(OC):
                for kp in range(KC // 2):
                    for h in range(RH):
                        nc.tensor.matmul(
                            acc[o][h][:],
                            wt[:, 2 * kp:2 * kp + 2, o * 128:(o + 1) * 128],
                            at[:, 2 * kp:2 * kp + 2, h * 512:(h + 1) * 512],
                            start=(w == 0 and kp == 0), stop=False,
                            perf_mode=mybir.MatmulPerfMode.DoubleRow)
            continue
        tiles = {}
        for nm, ap in ins.items():
            if nm in ("a8", "w8"):
                continue
            fd = ROWS if nm.startswith("a") else COUTS
            t = asb.tile([128, KC, fd], mdt, name=nm + "_t", tag=nm)
            eng = nc.sync if (nm.startswith("a") or W_QUEUE == "sync") else nc.scalar
            eng.dma_start(t[:], ap[w - fp8n])
            tiles[nm] = t
        if mode == "bf16x3":
            pairs = [(tiles["a_hi"], tiles["w_hi"]),
                     (tiles["a_hi"], tiles["w_lo"]),
                     (tiles["a_lo"], tiles["w_hi"])]
        else:
            pairs = [(tiles["a"], tiles["w"])]
        first_w, last_w = (w == 0), (w == KW - 1)
        for o in range(OC):
            for pi, (at, wt) in enumerate(pairs):
                for k in range(KC):
                    # One stationary load serves RH consecutive row-half
                    # matmuls (redundant Ldweights stripped post-compile).
                    for h in range(RH):
                        start = first_w and k == 0 and pi == 0
                        stop = (last_w and k == KC - 1 and pi == len(pairs) - 1)
                        nc.tensor.matmul(
                            acc[o][h][:],
                            wt[:, k, o * 128:(o + 1) * 128],
                            at[:, k, h * 512:(h + 1) * 512],
                            start=start, stop=stop)

    # Drain PSUM through the Activation engine, descaling the SA*SW
    # operand scale-up and adding bias (per-partition AP: partition p of
    # o-chunk o holds cout o*128+p). out = in*scale + bias.
    descale = 1.0 / (SA * SW) if mode == "bf16" else 1.0
    for o in range(OC):
        for h in range(RH):
            o_sb = osb.tile([128, 512], f32, tag="o")
            nc.scalar.activation(o_sb[:], acc[o][h][:],
                                 mybir.ActivationFunctionType.Identity,
                                 bias=bias_sb[:, o:o + 1], scale=descale)
            nc.sync.dma_start(
                out_t[o * 128:(o + 1) * 128, h * 512:(h + 1) * 512], o_sb[:])


_NC_CACHE = {}


def _get_nc(mode):
    if mode not in _NC_CACHE:
        _NC_CACHE[mode] = _build_nc(mode)
    return _NC_CACHE[mode]


def _pack_pmaj(x):
    """[KW, C_IN, fd] -> [KW, 128, KC*fd] so each partition line (k-major
    within a partition) is one contiguous DMA read."""
    kw, cin, fd = x.shape
    return np.ascontiguousarray(
        x.reshape(kw, KC, 128, fd).transpose(0, 2, 1, 3).reshape(kw, 128, KC * fd))


def _pack_inputs(in_, weight, bias, mode):
    """Host-side gather/transpose pack. Returns list of per-core input maps.
    Core id c = oq * NSH + nq."""
    import ml_dtypes

    in_ = np.asarray(in_, dtype=np.float32)
    weight = np.asarray(weight, dtype=np.float32)
    bias = np.asarray(bias, dtype=np.float32)

    # G[w, y, n, c] = in_[(y+1)(w+1)-1, n, c]
    G = in_[_XS.reshape(-1)].reshape(KW, Y_OUT, N_BATCH, C_IN)
    # A_all[w, c, y, n]
    A_all = np.ascontiguousarray(G.transpose(0, 3, 1, 2))

    def split(x):
        hi = x.astype(ml_dtypes.bfloat16)
        lo = (x - hi.astype(np.float32)).astype(ml_dtypes.bfloat16)
        return hi, lo

    fp8n = FP8_TAPS if mode == "bf16" else 0
    f8 = ml_dtypes.float8_e4m3

    # Per-nq A packs (shared across the CSH cores of each batch slice).
    # In bf16 mode every tap is pre-scaled by SA (power of 2, exact in
    # bf16) so fp8 and bf16 taps accumulate at one PSUM scale; the
    # device drain multiplies by 1/(SA*SW).
    a_packs = []
    for nq in range(NSH):
        n0 = nq * N_PER
        a_c = np.ascontiguousarray(
            A_all[:, :, :, n0:n0 + N_PER]).reshape(KW, C_IN, ROWS)
        if mode == "bf16x3":
            a_hi, a_lo = split(a_c)
            a_packs.append({"a_hi": _pack_pmaj(a_hi), "a_lo": _pack_pmaj(a_lo)})
        else:
            p = {"a": _pack_pmaj(
                (a_c[fp8n:] * SA).astype(ml_dtypes.bfloat16))}
            if fp8n:
                p["a8"] = _pack_pmaj((a_c[:fp8n] * SA).astype(f8))
            a_packs.append(p)

    # Per-oq W packs and bias packs.
    w_packs, b_packs = [], []
    for oq in range(CSH):
        o0 = oq * COUTS
        w_c = np.ascontiguousarray(weight[:, :, o0:o0 + COUTS])
        if mode == "bf16x3":
            w_hi, w_lo = split(w_c)
            w_packs.append({"w_hi": _pack_pmaj(w_hi), "w_lo": _pack_pmaj(w_lo)})
        else:
            p = {"w": _pack_pmaj(
                (w_c[fp8n:] * SW).astype(ml_dtypes.bfloat16))}
            if fp8n:
                p["w8"] = _pack_pmaj((w_c[:fp8n] * SW).astype(f8))
            w_packs.append(p)
        b_packs.append(np.ascontiguousarray(
            bias[o0:o0 + COUTS].reshape(OC, 128).T))

    in_maps = []
    for oq in range(CSH):
        for nq in range(NSH):
            m = {"bias": b_packs[oq]}
            m.update(a_packs[nq])
            m.update(w_packs[oq])
            in_maps.append(m)
    return in_maps


def kernel(in_, weight, bias):
    from concourse.bass_utils import run_bass_kernel_spmd

    nc = _get_nc(MODE)
    in_maps = _pack_inputs(in_, weight, bias, MODE)
    res = run_bass_kernel_spmd(nc, in_maps, core_ids=list(range(N_CORES)))
    # Core c = oq*NSH + nq returns out [COUTS, ROWS], rows = y*N_PER + n_local.
    out = np.empty((Y_OUT, N_BATCH, C_OUT), np.float32)
    for oq in range(CSH):
        for nq in range(NSH):
            blk = res.results[oq * NSH + nq]["out"].reshape(COUTS, Y_OUT, N_PER)
            out[:, nq * N_PER:(nq + 1) * N_PER,
                oq * COUTS:(oq + 1) * COUTS] = blk.transpose(1, 2, 0)
    return out
